# revision 3
# baseline (speedup 1.0000x reference)
"""DGCNN (5x EdgeConv + global-max MLP) Trainium2 Bass kernel, 8 NeuronCores.

Algebraic reduction: for EdgeConv with W=[Wtop;Wbot] ([2C,64]):
    msg_e = relu(x_i@Wtop + (x_j - x_i)@Wbot + b)
          = relu(a_i + y_j),  a = h@(Wtop-Wbot)+b,  y = h@Wbot
    h'_i  = max_e msg_e = relu(a_i + max_{j in N(i)} y_j)   (relu monotone)

Sharding: core r owns nodes [12500r, 12500(r+1)). Per layer: compute local
y rows, AllGather into a [100000,64] fp32 DRAM table, gather the 16 source
rows per node via canonical [128,1]-offset indirect DMAs, strided
reduce_max, fused-bias GEMM (ones column appended to features) + relu,
PE-transpose back into the resident transposed feature buffer.
"""
import numpy as np

import concourse.bass as bass
import concourse.bacc as bacc
import concourse.mybir as mybir
import concourse.tile as tile

N = 100000
K = 16
D = 64
NCORES = 8
NLOC = N // NCORES            # 12500
NT = (NLOC + 127) // 128      # 98 tiles
NPAD = NT * 128               # 12544
F32 = mybir.dt.float32
I32 = mybir.dt.int32

_CACHE = {}


def _build():
    nc = bacc.Bacc("TRN2", target_bir_lowering=False, debug=False,
                   num_devices=NCORES)
    x_loc = nc.dram_tensor("x_loc", [NPAD, 4], F32, kind="ExternalInput")
    idx_d = nc.dram_tensor("idx", [128, NT * K], I32, kind="ExternalInput")
    w1_d = nc.dram_tensor("W1", [6, D], F32, kind="ExternalInput")
    b1_d = nc.dram_tensor("b1", [D], F32, kind="ExternalInput")
    w_ds = [nc.dram_tensor(f"W{i}", [128, D], F32, kind="ExternalInput")
            for i in range(2, 6)]
    b_ds = [nc.dram_tensor(f"b{i}", [D], F32, kind="ExternalInput")
            for i in range(2, 6)]
    wa_d = nc.dram_tensor("Wa", [128, 128], F32, kind="ExternalInput")
    ba_d = nc.dram_tensor("ba", [128], F32, kind="ExternalInput")
    wb_d = nc.dram_tensor("Wb", [128, 3], F32, kind="ExternalInput")
    bb_d = nc.dram_tensor("bb", [3], F32, kind="ExternalInput")
    out_d = nc.dram_tensor("out", [NLOC, 3], F32, kind="ExternalOutput")

    cc_in = nc.dram_tensor("cc_in", [NLOC, D], F32, kind="Internal")
    table = nc.dram_tensor("table", [N, D], F32, kind="Internal",
                           addr_space="Shared")
    gm_in = nc.dram_tensor("gm_in", [D, 1], F32, kind="Internal")
    gm_out = nc.dram_tensor("gm_out", [D, 1], F32, kind="Internal",
                            addr_space="Shared")
    ident_d = nc.inline_tensor(np.eye(128, dtype=np.float32), name="ident")

    rg = [list(range(NCORES))]

    with tile.TileContext(nc) as tc:
        with (
            tc.tile_pool(name="const", bufs=1) as cst,
            tc.tile_pool(name="ht", bufs=1) as htp,
            tc.tile_pool(name="work", bufs=6) as wk,
            tc.tile_pool(name="psum", bufs=2, space="PSUM") as ps,
        ):
            ident = cst.tile([128, 128], F32, tag="ident")
            nc.sync.dma_start(ident[:], ident_d[:])
            idx_sb = cst.tile([128, NT * K], I32, tag="idx")
            nc.sync.dma_start(idx_sb[:], idx_d[:])

            # layer-1 weights: C=3 (+ ones row -> K=4)
            w1db = cst.tile([4, D], F32, tag="w1db")
            w1t = cst.tile([3, D], F32, tag="w1t")
            w1b = cst.tile([3, D], F32, tag="w1b")
            nc.sync.dma_start(w1t[:], w1_d[0:3, :])
            nc.sync.dma_start(w1b[:], w1_d[3:6, :])
            nc.vector.tensor_tensor(out=w1db[0:3, :], in0=w1t[:], in1=w1b[:],
                                    op=mybir.AluOpType.subtract)
            nc.sync.dma_start(w1db[3:4, :], b1_d[None, :])

            # layers 2-5 weights: C=64 (+ ones row -> K=65)
            wdbs, wbots = [], []
            for li in range(4):
                wt = cst.tile([D, D], F32, tag=f"wt{li}")
                wb = cst.tile([D, D], F32, tag=f"wb{li}")
                nc.sync.dma_start(wt[:], w_ds[li][0:D, :])
                nc.sync.dma_start(wb[:], w_ds[li][D:128, :])
                wdb = cst.tile([D + 1, D], F32, tag=f"wdb{li}")
                nc.vector.tensor_tensor(out=wdb[0:D, :], in0=wt[:], in1=wb[:],
                                        op=mybir.AluOpType.subtract)
                nc.sync.dma_start(wdb[D:D + 1, :], b_ds[li][None, :])
                wdbs.append(wdb)
                wbots.append(wb)

            wa_top = cst.tile([D, 128], F32, tag="wa_top")
            wa_bot = cst.tile([D, 128], F32, tag="wa_bot")
            nc.sync.dma_start(wa_top[:], wa_d[0:D, :])
            nc.sync.dma_start(wa_bot[:], wa_d[D:128, :])
            wb_sb = cst.tile([128, 3], F32, tag="wb_sb")
            nc.sync.dma_start(wb_sb[:], wb_d[:])
            ba_sb = cst.tile([128, 1], F32, tag="ba_sb")
            nc.sync.dma_start(ba_sb[:], ba_d[:, None])
            bb_rep = cst.tile([128, 3], F32, tag="bb_rep")
            nc.sync.dma_start(bb_rep[:], bb_d[None, :].to_broadcast([128, 3]))

            # resident transposed features (+ ones row at partition D)
            hts = [cst.tile([D + 1, NPAD], F32, tag="hTA", name="hTA"),
                   cst.tile([D + 1, NPAD], F32, tag="hTB", name="hTB")]
            nc.gpsimd.memset(hts[0][D:D + 1, :], 1.0)
            nc.gpsimd.memset(hts[1][D:D + 1, :], 1.0)

            def phase_a(layer):
                """local y rows -> cc_in (node-major)."""
                for t in range(NT):
                    p = min(128, NLOC - 128 * t)
                    if p <= 0:
                        break
                    if layer == 0:
                        x_sb = wk.tile([128, 4], F32, tag="xa")
                        nc.sync.dma_start(
                            x_sb[:], x_loc[128 * t:128 * (t + 1), :])
                        xt_ps = ps.tile([4, 128], F32, tag="tp", bufs=3,
                                        space="PSUM")
                        nc.tensor.transpose(out=xt_ps[:], in_=x_sb[:],
                                            identity=ident[:])
                        xt_sb = wk.tile([4, 128], F32, tag="xts")
                        nc.vector.tensor_copy(out=xt_sb[:], in_=xt_ps[:])
                        y_ps = ps.tile([128, D], F32, tag="mm", bufs=3, space="PSUM")
                        nc.tensor.matmul(out=y_ps[:], lhsT=xt_sb[0:3, :],
                                         rhs=w1b[:], start=True, stop=True)
                    else:
                        ht_in = hts[(layer + 1) % 2]
                        y_ps = ps.tile([128, D], F32, tag="mm", bufs=3, space="PSUM")
                        nc.tensor.matmul(
                            out=y_ps[:],
                            lhsT=ht_in[0:D, 128 * t:128 * (t + 1)],
                            rhs=wbots[layer - 1][:], start=True, stop=True)
                    y_sb = wk.tile([128, D], F32, tag="ysb")
                    nc.vector.tensor_copy(out=y_sb[:], in_=y_ps[:])
                    nc.sync.dma_start(cc_in[128 * t:128 * t + p, :],
                                      y_sb[:p, :])

            def phase_b(layer):
                """gather+max, fused-bias GEMM, relu, transpose into ht_out."""
                ht_out = hts[layer % 2]
                for t in range(NT):
                    gt = wk.tile([128, K, D], F32, tag="g")
                    for s in range(K):
                        c = K * t + s
                        nc.gpsimd.indirect_dma_start(
                            out=gt[:, s, :], out_offset=None, in_=table[:],
                            in_offset=bass.IndirectOffsetOnAxis(
                                ap=idx_sb[:, c:c + 1], axis=0))
                    m = wk.tile([128, D], F32, tag="m")
                    nc.vector.reduce_max(
                        m[:], gt[:].rearrange("p s f -> p f s"),
                        axis=mybir.AxisListType.X)
                    a_ps = ps.tile([128, D], F32, tag="mm", bufs=3, space="PSUM")
                    if layer == 0:
                        x_sb = wk.tile([128, 4], F32, tag="xb")
                        nc.sync.dma_start(
                            x_sb[:], x_loc[128 * t:128 * (t + 1), :])
                        xt_ps = ps.tile([4, 128], F32, tag="tp", bufs=3,
                                        space="PSUM")
                        nc.tensor.transpose(out=xt_ps[:], in_=x_sb[:],
                                            identity=ident[:])
                        xt_sb = wk.tile([4, 128], F32, tag="xtb")
                        nc.vector.tensor_copy(out=xt_sb[:], in_=xt_ps[:])
                        nc.tensor.matmul(out=a_ps[:], lhsT=xt_sb[:],
                                         rhs=w1db[:], start=True, stop=True)
                    else:
                        ht_in = hts[(layer + 1) % 2]
                        nc.tensor.matmul(
                            out=a_ps[:],
                            lhsT=ht_in[:, 128 * t:128 * (t + 1)],
                            rhs=wdbs[layer - 1][:], start=True, stop=True)
                    hpre = wk.tile([128, D], F32, tag="hp")
                    nc.vector.tensor_tensor(out=hpre[:], in0=a_ps[:],
                                            in1=m[:],
                                            op=mybir.AluOpType.add)
                    hrl = wk.tile([128, D], F32, tag="hr")
                    nc.scalar.activation(hrl[:], hpre[:],
                                         mybir.ActivationFunctionType.Relu)
                    tp_ps = ps.tile([D, 128], F32, tag="tp", bufs=3, space="PSUM")
                    nc.tensor.transpose(out=tp_ps[:], in_=hrl[:],
                                        identity=ident[:])
                    nc.vector.tensor_copy(
                        out=ht_out[0:D, 128 * t:128 * (t + 1)],
                        in_=tp_ps[:])

            for layer in range(5):
                phase_a(layer)
                nc.gpsimd.collective_compute(
                    "AllGather", mybir.AluOpType.bypass, replica_groups=rg,
                    ins=[cc_in[:]], outs=[table[:]])
                phase_b(layer)

            # ---- final: global max, MLP head, residual ----
            ht5 = hts[0]  # layer index 4 -> hts[4 % 2] = hts[0]
            gm_sb = wk.tile([D, 1], F32, tag="gm")
            nc.vector.reduce_max(gm_sb[:], ht5[0:D, 0:NLOC],
                                 axis=mybir.AxisListType.X)
            nc.sync.dma_start(gm_in[:], gm_sb[:])
            nc.gpsimd.collective_compute(
                "AllReduce", mybir.AluOpType.max, replica_groups=rg,
                ins=[gm_in[:]], outs=[gm_out[:]])
            gmr = wk.tile([D, 1], F32, tag="gmr")
            nc.sync.dma_start(gmr[:], gm_out[:])
            q_ps = ps.tile([128, 1], F32, tag="mm", bufs=3, space="PSUM")
            nc.tensor.matmul(out=q_ps[:], lhsT=wa_bot[:], rhs=gmr[:],
                             start=True, stop=True)
            bias_vec = cst.tile([128, 1], F32, tag="bias_vec")
            nc.vector.tensor_tensor(out=bias_vec[:], in0=q_ps[:],
                                    in1=ba_sb[:], op=mybir.AluOpType.add)

            NCHUNK = 512
            for c0 in range(0, NLOC, NCHUNK):
                cw = min(NCHUNK, NLOC - c0)
                f_ps = ps.tile([128, NCHUNK], F32, tag="fps", bufs=2, space="PSUM")
                nc.tensor.matmul(out=f_ps[:, :cw], lhsT=wa_top[:],
                                 rhs=ht5[0:D, c0:c0 + cw],
                                 start=True, stop=True)
                f_sb = wk.tile([128, NCHUNK], F32, tag="fsb")
                nc.scalar.activation(f_sb[:, :cw], f_ps[:, :cw],
                                     mybir.ActivationFunctionType.Relu,
                                     bias=bias_vec[:])
                for s0 in range(0, cw, 128):
                    sw = min(128, cw - s0)
                    o_ps = ps.tile([128, 3], F32, tag="mm", bufs=3, space="PSUM")
                    nc.tensor.matmul(out=o_ps[:sw, :],
                                     lhsT=f_sb[:, s0:s0 + sw],
                                     rhs=wb_sb[:], start=True, stop=True)
                    x_sb = wk.tile([128, 4], F32, tag="xf")
                    nc.sync.dma_start(
                        x_sb[:sw, :], x_loc[c0 + s0:c0 + s0 + sw, :])
                    o1 = wk.tile([128, 3], F32, tag="o1")
                    nc.vector.tensor_tensor(out=o1[:sw, :], in0=o_ps[:sw, :],
                                            in1=bb_rep[:sw, :],
                                            op=mybir.AluOpType.add)
                    o2 = wk.tile([128, 3], F32, tag="o2")
                    nc.vector.tensor_tensor(out=o2[:sw, :], in0=o1[:sw, :],
                                            in1=x_sb[:sw, 0:3],
                                            op=mybir.AluOpType.add)
                    nc.sync.dma_start(out_d[c0 + s0:c0 + s0 + sw, :],
                                      o2[:sw, :])
    nc.compile()
    return nc


def _get_nc():
    if "nc" not in _CACHE:
        _CACHE["nc"] = _build()
    return _CACHE["nc"]


def _prep_in_maps(x, edge_index, W1, b1, W2, b2, W3, b3, W4, b4, W5, b5,
                  Wa, ba, Wb, bb):
    x = np.asarray(x, dtype=np.float32)
    src = np.asarray(edge_index[0]).astype(np.int32).reshape(N, K)
    dst = np.asarray(edge_index[1]).astype(np.int64)
    assert np.array_equal(dst, np.repeat(np.arange(N, dtype=np.int64), K)), \
        "kernel assumes dst = repeat(arange(N), K)"
    common = {
        "W1": np.asarray(W1, np.float32), "b1": np.asarray(b1, np.float32),
        "W2": np.asarray(W2, np.float32), "b2": np.asarray(b2, np.float32),
        "W3": np.asarray(W3, np.float32), "b3": np.asarray(b3, np.float32),
        "W4": np.asarray(W4, np.float32), "b4": np.asarray(b4, np.float32),
        "W5": np.asarray(W5, np.float32), "b5": np.asarray(b5, np.float32),
        "Wa": np.asarray(Wa, np.float32), "ba": np.asarray(ba, np.float32),
        "Wb": np.asarray(Wb, np.float32), "bb": np.asarray(bb, np.float32),
    }
    in_maps = []
    for r in range(NCORES):
        lo = NLOC * r
        xp = np.zeros((NPAD, 4), dtype=np.float32)
        xp[:NLOC, 0:3] = x[lo:lo + NLOC]
        xp[:, 3] = 1.0
        sp = np.zeros((NPAD, K), dtype=np.int32)
        sp[:NLOC] = src[lo:lo + NLOC]
        idx_arr = np.zeros((128, NT * K), dtype=np.int32)
        for t in range(NT):
            idx_arr[:, K * t:K * (t + 1)] = sp[128 * t:128 * (t + 1)]
        in_maps.append({"x_loc": xp, "idx": idx_arr, **common})
    return in_maps


def kernel(**inputs) -> np.ndarray:
    from concourse.bass_utils import run_bass_kernel_spmd
    nc = _get_nc()
    in_maps = _prep_in_maps(**inputs)
    res = run_bass_kernel_spmd(nc, in_maps, core_ids=list(range(NCORES)))
    return np.concatenate([res.results[r]["out"] for r in range(NCORES)],
                          axis=0)


# revision 4
# speedup vs baseline: 12.1576x; 12.1576x over previous
"""DGCNN (5x EdgeConv + global-max MLP) Trainium2 Bass kernel, 8 NeuronCores.

Algebraic reduction: for EdgeConv with W=[Wtop;Wbot] ([2C,64]):
    msg_e = relu(x_i@Wtop + (x_j - x_i)@Wbot + b)
          = relu(a_i + y_j),  a = h@(Wtop-Wbot)+b,  y = h@Wbot
    h'_i  = max_e msg_e = relu(a_i + max_{j in N(i)} y_j)   (relu monotone)

Sharding: core r owns nodes [12500r, 12500(r+1)). Per layer: compute local
y rows, AllGather into a [100000,64] fp32 DRAM table, gather the 16 source
rows per node via canonical [128,1]-offset indirect DMAs, strided
reduce_max, fused-bias GEMM (ones column appended to features) + relu,
PE-transpose back into the resident transposed feature buffer.
"""
import numpy as np

import concourse.bass as bass
import concourse.bacc as bacc
import concourse.mybir as mybir
import concourse.tile as tile

N = 100000
K = 16
D = 64
NCORES = 8
NLOC = N // NCORES            # 12500
NT = (NLOC + 127) // 128      # 98 tiles
NPAD = NT * 128               # 12544
F32 = mybir.dt.float32
I32 = mybir.dt.int32

_CACHE = {}


def _build(reps: int = 1):
    nc = bacc.Bacc("TRN2", target_bir_lowering=False, debug=False,
                   num_devices=NCORES)
    x_loc = nc.dram_tensor("x_loc", [NPAD, 4], F32, kind="ExternalInput")
    idx_d = nc.dram_tensor("idx", [128, NT * K], I32, kind="ExternalInput")
    w1_d = nc.dram_tensor("W1", [6, D], F32, kind="ExternalInput")
    b1_d = nc.dram_tensor("b1", [D], F32, kind="ExternalInput")
    w_ds = [nc.dram_tensor(f"W{i}", [128, D], F32, kind="ExternalInput")
            for i in range(2, 6)]
    b_ds = [nc.dram_tensor(f"b{i}", [D], F32, kind="ExternalInput")
            for i in range(2, 6)]
    wa_d = nc.dram_tensor("Wa", [128, 128], F32, kind="ExternalInput")
    ba_d = nc.dram_tensor("ba", [128], F32, kind="ExternalInput")
    wb_d = nc.dram_tensor("Wb", [128, 3], F32, kind="ExternalInput")
    bb_d = nc.dram_tensor("bb", [3], F32, kind="ExternalInput")
    out_d = nc.dram_tensor("out", [NLOC, 3], F32, kind="ExternalOutput")

    cc_in = nc.dram_tensor("cc_in", [NLOC, D], F32, kind="Internal")
    table = nc.dram_tensor("table", [N, D], F32, kind="Internal",
                           addr_space="Shared")
    gm_in = nc.dram_tensor("gm_in", [D, 1], F32, kind="Internal")
    gm_out = nc.dram_tensor("gm_out", [D, 1], F32, kind="Internal",
                            addr_space="Shared")
    ident_d = nc.inline_tensor(np.eye(128, dtype=np.float32), name="ident")

    rg = [list(range(NCORES))]

    with tile.TileContext(nc) as tc:
        with (
            tc.tile_pool(name="const", bufs=1) as cst,
            tc.tile_pool(name="ht", bufs=1) as htp,
            tc.tile_pool(name="work", bufs=6) as wk,
            tc.tile_pool(name="psum", bufs=2, space="PSUM") as ps,
        ):
            ident = cst.tile([128, 128], F32, tag="ident")
            nc.sync.dma_start(ident[:], ident_d[:])
            idx_sb = cst.tile([128, NT * K], I32, tag="idx")
            nc.sync.dma_start(idx_sb[:], idx_d[:])

            # layer-1 weights: C=3 (+ ones row -> K=4)
            w1db = cst.tile([4, D], F32, tag="w1db")
            w1t = cst.tile([3, D], F32, tag="w1t")
            w1b = cst.tile([3, D], F32, tag="w1b")
            nc.sync.dma_start(w1t[:], w1_d[0:3, :])
            nc.sync.dma_start(w1b[:], w1_d[3:6, :])
            nc.vector.tensor_tensor(out=w1db[0:3, :], in0=w1t[:], in1=w1b[:],
                                    op=mybir.AluOpType.subtract)
            nc.sync.dma_start(w1db[3:4, :], b1_d[None, :])

            # layers 2-5 weights: C=64 (+ ones row -> K=65)
            wdbs, wbots = [], []
            for li in range(4):
                wt = cst.tile([D, D], F32, tag=f"wt{li}")
                wb = cst.tile([D, D], F32, tag=f"wb{li}")
                nc.sync.dma_start(wt[:], w_ds[li][0:D, :])
                nc.sync.dma_start(wb[:], w_ds[li][D:128, :])
                wdb = cst.tile([D + 1, D], F32, tag=f"wdb{li}")
                nc.vector.tensor_tensor(out=wdb[0:D, :], in0=wt[:], in1=wb[:],
                                        op=mybir.AluOpType.subtract)
                nc.sync.dma_start(wdb[D:D + 1, :], b_ds[li][None, :])
                wdbs.append(wdb)
                wbots.append(wb)

            wa_top = cst.tile([D, 128], F32, tag="wa_top")
            wa_bot = cst.tile([D, 128], F32, tag="wa_bot")
            nc.sync.dma_start(wa_top[:], wa_d[0:D, :])
            nc.sync.dma_start(wa_bot[:], wa_d[D:128, :])
            wb_sb = cst.tile([128, 3], F32, tag="wb_sb")
            nc.sync.dma_start(wb_sb[:], wb_d[:])
            ba_sb = cst.tile([128, 1], F32, tag="ba_sb")
            nc.sync.dma_start(ba_sb[:], ba_d[:, None])
            bb_rep = cst.tile([128, 3], F32, tag="bb_rep")
            nc.sync.dma_start(bb_rep[:], bb_d[None, :].to_broadcast([128, 3]))

            # resident transposed features (+ ones row at partition D)
            hts = [cst.tile([D + 1, NPAD], F32, tag="hTA", name="hTA"),
                   cst.tile([D + 1, NPAD], F32, tag="hTB", name="hTB")]
            nc.gpsimd.memset(hts[0][D:D + 1, :], 1.0)
            nc.gpsimd.memset(hts[1][D:D + 1, :], 1.0)

            def phase_a(layer):
                """local y rows -> cc_in (node-major)."""
                for t in range(NT):
                    p = min(128, NLOC - 128 * t)
                    if p <= 0:
                        break
                    if layer == 0:
                        x_sb = wk.tile([128, 4], F32, tag="xa")
                        nc.sync.dma_start(
                            x_sb[:], x_loc[128 * t:128 * (t + 1), :])
                        xt_ps = ps.tile([4, 128], F32, tag="tp", bufs=3,
                                        space="PSUM")
                        nc.tensor.transpose(out=xt_ps[:], in_=x_sb[:],
                                            identity=ident[:])
                        xt_sb = wk.tile([4, 128], F32, tag="xts")
                        nc.vector.tensor_copy(out=xt_sb[:], in_=xt_ps[:])
                        y_ps = ps.tile([128, D], F32, tag="mm", bufs=3, space="PSUM")
                        nc.tensor.matmul(out=y_ps[:], lhsT=xt_sb[0:3, :],
                                         rhs=w1b[:], start=True, stop=True)
                    else:
                        ht_in = hts[(layer + 1) % 2]
                        y_ps = ps.tile([128, D], F32, tag="mm", bufs=3, space="PSUM")
                        nc.tensor.matmul(
                            out=y_ps[:],
                            lhsT=ht_in[0:D, 128 * t:128 * (t + 1)],
                            rhs=wbots[layer - 1][:], start=True, stop=True)
                    y_sb = wk.tile([128, D], F32, tag="ysb")
                    nc.vector.tensor_copy(out=y_sb[:], in_=y_ps[:])
                    nc.sync.dma_start(cc_in[128 * t:128 * t + p, :],
                                      y_sb[:p, :])

            def phase_b(layer):
                """gather+max, fused-bias GEMM, relu, transpose into ht_out."""
                ht_out = hts[layer % 2]
                for t in range(NT):
                    gt = wk.tile([128, K, D], F32, tag="g")
                    for s in range(K):
                        c = K * t + s
                        nc.gpsimd.indirect_dma_start(
                            out=gt[:, s, :], out_offset=None, in_=table[:],
                            in_offset=bass.IndirectOffsetOnAxis(
                                ap=idx_sb[:, c:c + 1], axis=0))
                    m = wk.tile([128, D], F32, tag="m")
                    nc.vector.reduce_max(
                        m[:], gt[:].rearrange("p s f -> p f s"),
                        axis=mybir.AxisListType.X)
                    a_ps = ps.tile([128, D], F32, tag="mm", bufs=3, space="PSUM")
                    if layer == 0:
                        x_sb = wk.tile([128, 4], F32, tag="xb")
                        nc.sync.dma_start(
                            x_sb[:], x_loc[128 * t:128 * (t + 1), :])
                        xt_ps = ps.tile([4, 128], F32, tag="tp", bufs=3,
                                        space="PSUM")
                        nc.tensor.transpose(out=xt_ps[:], in_=x_sb[:],
                                            identity=ident[:])
                        xt_sb = wk.tile([4, 128], F32, tag="xtb")
                        nc.vector.tensor_copy(out=xt_sb[:], in_=xt_ps[:])
                        nc.tensor.matmul(out=a_ps[:], lhsT=xt_sb[:],
                                         rhs=w1db[:], start=True, stop=True)
                    else:
                        ht_in = hts[(layer + 1) % 2]
                        nc.tensor.matmul(
                            out=a_ps[:],
                            lhsT=ht_in[:, 128 * t:128 * (t + 1)],
                            rhs=wdbs[layer - 1][:], start=True, stop=True)
                    hpre = wk.tile([128, D], F32, tag="hp")
                    nc.vector.tensor_tensor(out=hpre[:], in0=a_ps[:],
                                            in1=m[:],
                                            op=mybir.AluOpType.add)
                    hrl = wk.tile([128, D], F32, tag="hr")
                    nc.scalar.activation(hrl[:], hpre[:],
                                         mybir.ActivationFunctionType.Relu)
                    tp_ps = ps.tile([D, 128], F32, tag="tp", bufs=3, space="PSUM")
                    nc.tensor.transpose(out=tp_ps[:], in_=hrl[:],
                                        identity=ident[:])
                    nc.vector.tensor_copy(
                        out=ht_out[0:D, 128 * t:128 * (t + 1)],
                        in_=tp_ps[:])

            for _rep in range(reps):
              for layer in range(5):
                phase_a(layer)
                nc.gpsimd.collective_compute(
                    "AllGather", mybir.AluOpType.bypass, replica_groups=rg,
                    ins=[cc_in[:]], outs=[table[:]])
                phase_b(layer)

            # ---- final: global max, MLP head, residual ----
            ht5 = hts[0]  # layer index 4 -> hts[4 % 2] = hts[0]
            gm_sb = wk.tile([D, 1], F32, tag="gm")
            nc.vector.reduce_max(gm_sb[:], ht5[0:D, 0:NLOC],
                                 axis=mybir.AxisListType.X)
            nc.sync.dma_start(gm_in[:], gm_sb[:])
            nc.gpsimd.collective_compute(
                "AllReduce", mybir.AluOpType.max, replica_groups=rg,
                ins=[gm_in[:]], outs=[gm_out[:]])
            gmr = wk.tile([D, 1], F32, tag="gmr")
            nc.sync.dma_start(gmr[:], gm_out[:])
            q_ps = ps.tile([128, 1], F32, tag="mm", bufs=3, space="PSUM")
            nc.tensor.matmul(out=q_ps[:], lhsT=wa_bot[:], rhs=gmr[:],
                             start=True, stop=True)
            bias_vec = cst.tile([128, 1], F32, tag="bias_vec")
            nc.vector.tensor_tensor(out=bias_vec[:], in0=q_ps[:],
                                    in1=ba_sb[:], op=mybir.AluOpType.add)

            NCHUNK = 512
            for c0 in range(0, NLOC, NCHUNK):
                cw = min(NCHUNK, NLOC - c0)
                f_ps = ps.tile([128, NCHUNK], F32, tag="fps", bufs=2, space="PSUM")
                nc.tensor.matmul(out=f_ps[:, :cw], lhsT=wa_top[:],
                                 rhs=ht5[0:D, c0:c0 + cw],
                                 start=True, stop=True)
                f_sb = wk.tile([128, NCHUNK], F32, tag="fsb")
                nc.scalar.activation(f_sb[:, :cw], f_ps[:, :cw],
                                     mybir.ActivationFunctionType.Relu,
                                     bias=bias_vec[:])
                for s0 in range(0, cw, 128):
                    sw = min(128, cw - s0)
                    o_ps = ps.tile([128, 3], F32, tag="mm", bufs=3, space="PSUM")
                    nc.tensor.matmul(out=o_ps[:sw, :],
                                     lhsT=f_sb[:, s0:s0 + sw],
                                     rhs=wb_sb[:], start=True, stop=True)
                    x_sb = wk.tile([128, 4], F32, tag="xf")
                    nc.sync.dma_start(
                        x_sb[:sw, :], x_loc[c0 + s0:c0 + s0 + sw, :])
                    o1 = wk.tile([128, 3], F32, tag="o1")
                    nc.vector.tensor_tensor(out=o1[:sw, :], in0=o_ps[:sw, :],
                                            in1=bb_rep[:sw, :],
                                            op=mybir.AluOpType.add)
                    o2 = wk.tile([128, 3], F32, tag="o2")
                    nc.vector.tensor_tensor(out=o2[:sw, :], in0=o1[:sw, :],
                                            in1=x_sb[:sw, 0:3],
                                            op=mybir.AluOpType.add)
                    nc.sync.dma_start(out_d[c0 + s0:c0 + s0 + sw, :],
                                      o2[:sw, :])
    nc.compile()
    return nc


def _get_nc():
    if "nc" not in _CACHE:
        _CACHE["nc"] = _build()
    return _CACHE["nc"]


def _prep_in_maps(x, edge_index, W1, b1, W2, b2, W3, b3, W4, b4, W5, b5,
                  Wa, ba, Wb, bb):
    x = np.asarray(x, dtype=np.float32)
    src = np.asarray(edge_index[0]).astype(np.int32).reshape(N, K)
    dst = np.asarray(edge_index[1]).astype(np.int64)
    assert np.array_equal(dst, np.repeat(np.arange(N, dtype=np.int64), K)), \
        "kernel assumes dst = repeat(arange(N), K)"
    common = {
        "W1": np.asarray(W1, np.float32), "b1": np.asarray(b1, np.float32),
        "W2": np.asarray(W2, np.float32), "b2": np.asarray(b2, np.float32),
        "W3": np.asarray(W3, np.float32), "b3": np.asarray(b3, np.float32),
        "W4": np.asarray(W4, np.float32), "b4": np.asarray(b4, np.float32),
        "W5": np.asarray(W5, np.float32), "b5": np.asarray(b5, np.float32),
        "Wa": np.asarray(Wa, np.float32), "ba": np.asarray(ba, np.float32),
        "Wb": np.asarray(Wb, np.float32), "bb": np.asarray(bb, np.float32),
    }
    in_maps = []
    for r in range(NCORES):
        lo = NLOC * r
        xp = np.zeros((NPAD, 4), dtype=np.float32)
        xp[:NLOC, 0:3] = x[lo:lo + NLOC]
        xp[:, 3] = 1.0
        sp = np.zeros((NPAD, K), dtype=np.int32)
        sp[:NLOC] = src[lo:lo + NLOC]
        idx_arr = np.zeros((128, NT * K), dtype=np.int32)
        for t in range(NT):
            idx_arr[:, K * t:K * (t + 1)] = sp[128 * t:128 * (t + 1)]
        in_maps.append({"x_loc": xp, "idx": idx_arr, **common})
    return in_maps


def kernel(**inputs) -> np.ndarray:
    from concourse.bass_utils import run_bass_kernel_spmd
    nc = _get_nc()
    in_maps = _prep_in_maps(**inputs)
    res = run_bass_kernel_spmd(nc, in_maps, core_ids=list(range(NCORES)))
    return np.concatenate([res.results[r]["out"] for r in range(NCORES)],
                          axis=0)


# revision 5
# speedup vs baseline: 16.3873x; 1.3479x over previous
"""DGCNN (5x EdgeConv + global-max MLP) Trainium2 Bass kernel, 8 NeuronCores.

Algebraic reduction: for EdgeConv with W=[Wtop;Wbot] ([2C,64]):
    msg_e = relu(x_i@Wtop + (x_j - x_i)@Wbot + b)
          = relu(a_i + y_j),  a = h@(Wtop-Wbot)+b,  y = h@Wbot
    h'_i  = max_e msg_e = relu(a_i + max_{j in N(i)} y_j)   (relu monotone)

Sharding: core r owns nodes [12500r, 12500(r+1)). Per layer: compute local
y rows, AllGather into a [100000,64] fp32 DRAM table, gather the 16 source
rows per node via canonical [128,1]-offset indirect DMAs, strided
reduce_max, fused-bias GEMM (ones column appended to features) + relu,
PE-transpose back into the resident transposed feature buffer.
"""
import numpy as np

import concourse.bass as bass
import concourse.bacc as bacc
import concourse.mybir as mybir
import concourse.tile as tile

N = 100000
K = 16
D = 64
NCORES = 8
NLOC = N // NCORES            # 12500
NT = (NLOC + 127) // 128      # 98 tiles
NPAD = NT * 128               # 12544
F32 = mybir.dt.float32
I32 = mybir.dt.int32

_CACHE = {}


def _build(reps: int = 1):
    nc = bacc.Bacc("TRN2", target_bir_lowering=False, debug=False,
                   num_devices=NCORES)
    x_loc = nc.dram_tensor("x_loc", [NPAD, 4], F32, kind="ExternalInput")
    idx_d = nc.dram_tensor("idx", [128, NT * K], I32, kind="ExternalInput")
    w1_d = nc.dram_tensor("W1", [6, D], F32, kind="ExternalInput")
    b1_d = nc.dram_tensor("b1", [D], F32, kind="ExternalInput")
    w_ds = [nc.dram_tensor(f"W{i}", [128, D], F32, kind="ExternalInput")
            for i in range(2, 6)]
    b_ds = [nc.dram_tensor(f"b{i}", [D], F32, kind="ExternalInput")
            for i in range(2, 6)]
    wa_d = nc.dram_tensor("Wa", [128, 128], F32, kind="ExternalInput")
    ba_d = nc.dram_tensor("ba", [128], F32, kind="ExternalInput")
    wb_d = nc.dram_tensor("Wb", [128, 3], F32, kind="ExternalInput")
    bb_d = nc.dram_tensor("bb", [3], F32, kind="ExternalInput")
    out_d = nc.dram_tensor("out", [NLOC, 3], F32, kind="ExternalOutput")

    cc_in = nc.dram_tensor("cc_in", [NLOC, D], F32, kind="Internal")
    table = nc.dram_tensor("table", [N, D], F32, kind="Internal",
                           addr_space="Shared")
    gm_in = nc.dram_tensor("gm_in", [D, 1], F32, kind="Internal")
    gm_out = nc.dram_tensor("gm_out", [D, 1], F32, kind="Internal",
                            addr_space="Shared")
    ident_d = nc.inline_tensor(np.eye(128, dtype=np.float32), name="ident")

    rg = [list(range(NCORES))]

    with tile.TileContext(nc) as tc:
        with (
            tc.tile_pool(name="const", bufs=1) as cst,
            tc.tile_pool(name="ht", bufs=1) as htp,
            tc.tile_pool(name="work", bufs=6) as wk,
            tc.tile_pool(name="psum", bufs=2, space="PSUM") as ps,
        ):
            ident = cst.tile([128, 128], F32, tag="ident")
            nc.sync.dma_start(ident[:], ident_d[:])
            idx_sb = cst.tile([128, NT * K], I32, tag="idx")
            nc.sync.dma_start(idx_sb[:], idx_d[:])

            # layer-1 weights: C=3 (+ ones row -> K=4)
            w1db = cst.tile([4, D], F32, tag="w1db")
            w1t = cst.tile([3, D], F32, tag="w1t")
            w1b = cst.tile([3, D], F32, tag="w1b")
            nc.sync.dma_start(w1t[:], w1_d[0:3, :])
            nc.sync.dma_start(w1b[:], w1_d[3:6, :])
            nc.vector.tensor_tensor(out=w1db[0:3, :], in0=w1t[:], in1=w1b[:],
                                    op=mybir.AluOpType.subtract)
            nc.sync.dma_start(w1db[3:4, :], b1_d[None, :])

            # layers 2-5 weights: C=64 (+ ones row -> K=65)
            wdbs, wbots = [], []
            for li in range(4):
                wt = cst.tile([D, D], F32, tag=f"wt{li}")
                wb = cst.tile([D, D], F32, tag=f"wb{li}")
                nc.sync.dma_start(wt[:], w_ds[li][0:D, :])
                nc.sync.dma_start(wb[:], w_ds[li][D:128, :])
                wdb = cst.tile([D + 1, D], F32, tag=f"wdb{li}")
                nc.vector.tensor_tensor(out=wdb[0:D, :], in0=wt[:], in1=wb[:],
                                        op=mybir.AluOpType.subtract)
                nc.sync.dma_start(wdb[D:D + 1, :], b_ds[li][None, :])
                wdbs.append(wdb)
                wbots.append(wb)

            wa_top = cst.tile([D, 128], F32, tag="wa_top")
            wa_bot = cst.tile([D, 128], F32, tag="wa_bot")
            nc.sync.dma_start(wa_top[:], wa_d[0:D, :])
            nc.sync.dma_start(wa_bot[:], wa_d[D:128, :])
            wb_sb = cst.tile([128, 3], F32, tag="wb_sb")
            nc.sync.dma_start(wb_sb[:], wb_d[:])
            ba_sb = cst.tile([128, 1], F32, tag="ba_sb")
            nc.sync.dma_start(ba_sb[:], ba_d[:, None])
            bb_rep = cst.tile([128, 3], F32, tag="bb_rep")
            nc.sync.dma_start(bb_rep[:], bb_d[None, :].to_broadcast([128, 3]))

            # resident transposed features (+ ones row at partition D)
            hts = [cst.tile([D + 1, NPAD], F32, tag="hTA", name="hTA"),
                   cst.tile([D + 1, NPAD], F32, tag="hTB", name="hTB")]
            nc.gpsimd.memset(hts[0][D:D + 1, :], 1.0)
            nc.gpsimd.memset(hts[1][D:D + 1, :], 1.0)

            def phase_a(layer):
                """local y rows -> cc_in (node-major)."""
                for t in range(NT):
                    p = min(128, NLOC - 128 * t)
                    if p <= 0:
                        break
                    if layer == 0:
                        x_sb = wk.tile([128, 4], F32, tag="xa")
                        nc.sync.dma_start(
                            x_sb[:], x_loc[128 * t:128 * (t + 1), :])
                        xt_ps = ps.tile([4, 128], F32, tag="tp", bufs=3,
                                        space="PSUM")
                        nc.tensor.transpose(out=xt_ps[:], in_=x_sb[:],
                                            identity=ident[:])
                        xt_sb = wk.tile([4, 128], F32, tag="xts")
                        nc.vector.tensor_copy(out=xt_sb[:], in_=xt_ps[:])
                        y_ps = ps.tile([128, D], F32, tag="mm", bufs=3, space="PSUM")
                        nc.tensor.matmul(out=y_ps[:], lhsT=xt_sb[0:3, :],
                                         rhs=w1b[:], start=True, stop=True)
                    else:
                        ht_in = hts[(layer + 1) % 2]
                        y_ps = ps.tile([128, D], F32, tag="mm", bufs=3, space="PSUM")
                        nc.tensor.matmul(
                            out=y_ps[:],
                            lhsT=ht_in[0:D, 128 * t:128 * (t + 1)],
                            rhs=wbots[layer - 1][:], start=True, stop=True)
                    y_sb = wk.tile([128, D], F32, tag="ysb")
                    nc.vector.tensor_copy(out=y_sb[:], in_=y_ps[:])
                    nc.sync.dma_start(cc_in[128 * t:128 * t + p, :],
                                      y_sb[:p, :])

            def phase_b(layer):
                """gather+max, fused-bias GEMM, relu, transpose into ht_out."""
                ht_out = hts[layer % 2]
                for t in range(NT):
                    gt = wk.tile([128, K, D], F32, tag="g", bufs=10)
                    for s in range(K):
                        c = K * t + s
                        nc.gpsimd.indirect_dma_start(
                            out=gt[:, s, :], out_offset=None, in_=table[:],
                            in_offset=bass.IndirectOffsetOnAxis(
                                ap=idx_sb[:, c:c + 1], axis=0))
                    m = wk.tile([128, D], F32, tag="m", bufs=8)
                    nc.vector.reduce_max(
                        m[:], gt[:].rearrange("p s f -> p f s"),
                        axis=mybir.AxisListType.X)
                    a_ps = ps.tile([128, D], F32, tag="mm", bufs=3, space="PSUM")
                    if layer == 0:
                        x_sb = wk.tile([128, 4], F32, tag="xb")
                        nc.sync.dma_start(
                            x_sb[:], x_loc[128 * t:128 * (t + 1), :])
                        xt_ps = ps.tile([4, 128], F32, tag="tp", bufs=3,
                                        space="PSUM")
                        nc.tensor.transpose(out=xt_ps[:], in_=x_sb[:],
                                            identity=ident[:])
                        xt_sb = wk.tile([4, 128], F32, tag="xtb")
                        nc.vector.tensor_copy(out=xt_sb[:], in_=xt_ps[:])
                        nc.tensor.matmul(out=a_ps[:], lhsT=xt_sb[:],
                                         rhs=w1db[:], start=True, stop=True)
                    else:
                        ht_in = hts[(layer + 1) % 2]
                        nc.tensor.matmul(
                            out=a_ps[:],
                            lhsT=ht_in[:, 128 * t:128 * (t + 1)],
                            rhs=wdbs[layer - 1][:], start=True, stop=True)
                    hpre = wk.tile([128, D], F32, tag="hp")
                    nc.vector.tensor_tensor(out=hpre[:], in0=a_ps[:],
                                            in1=m[:],
                                            op=mybir.AluOpType.add)
                    hrl = wk.tile([128, D], F32, tag="hr")
                    nc.scalar.activation(hrl[:], hpre[:],
                                         mybir.ActivationFunctionType.Relu)
                    tp_ps = ps.tile([D, 128], F32, tag="tp", bufs=3, space="PSUM")
                    nc.tensor.transpose(out=tp_ps[:], in_=hrl[:],
                                        identity=ident[:])
                    nc.vector.tensor_copy(
                        out=ht_out[0:D, 128 * t:128 * (t + 1)],
                        in_=tp_ps[:])

            for _rep in range(reps):
              for layer in range(5):
                phase_a(layer)
                nc.gpsimd.collective_compute(
                    "AllGather", mybir.AluOpType.bypass, replica_groups=rg,
                    ins=[cc_in[:]], outs=[table[:]])
                phase_b(layer)

            # ---- final: global max, MLP head, residual ----
            ht5 = hts[0]  # layer index 4 -> hts[4 % 2] = hts[0]
            gm_sb = wk.tile([D, 1], F32, tag="gm")
            nc.vector.reduce_max(gm_sb[:], ht5[0:D, 0:NLOC],
                                 axis=mybir.AxisListType.X)
            nc.sync.dma_start(gm_in[:], gm_sb[:])
            nc.gpsimd.collective_compute(
                "AllReduce", mybir.AluOpType.max, replica_groups=rg,
                ins=[gm_in[:]], outs=[gm_out[:]])
            gmr = wk.tile([D, 1], F32, tag="gmr")
            nc.sync.dma_start(gmr[:], gm_out[:])
            q_ps = ps.tile([128, 1], F32, tag="mm", bufs=3, space="PSUM")
            nc.tensor.matmul(out=q_ps[:], lhsT=wa_bot[:], rhs=gmr[:],
                             start=True, stop=True)
            bias_vec = cst.tile([128, 1], F32, tag="bias_vec")
            nc.vector.tensor_tensor(out=bias_vec[:], in0=q_ps[:],
                                    in1=ba_sb[:], op=mybir.AluOpType.add)

            NCHUNK = 512
            for c0 in range(0, NLOC, NCHUNK):
                cw = min(NCHUNK, NLOC - c0)
                f_ps = ps.tile([128, NCHUNK], F32, tag="fps", bufs=2, space="PSUM")
                nc.tensor.matmul(out=f_ps[:, :cw], lhsT=wa_top[:],
                                 rhs=ht5[0:D, c0:c0 + cw],
                                 start=True, stop=True)
                f_sb = wk.tile([128, NCHUNK], F32, tag="fsb")
                nc.scalar.activation(f_sb[:, :cw], f_ps[:, :cw],
                                     mybir.ActivationFunctionType.Relu,
                                     bias=bias_vec[:])
                for s0 in range(0, cw, 128):
                    sw = min(128, cw - s0)
                    o_ps = ps.tile([128, 3], F32, tag="mm", bufs=3, space="PSUM")
                    nc.tensor.matmul(out=o_ps[:sw, :],
                                     lhsT=f_sb[:, s0:s0 + sw],
                                     rhs=wb_sb[:], start=True, stop=True)
                    x_sb = wk.tile([128, 4], F32, tag="xf")
                    nc.sync.dma_start(
                        x_sb[:sw, :], x_loc[c0 + s0:c0 + s0 + sw, :])
                    o1 = wk.tile([128, 3], F32, tag="o1")
                    nc.vector.tensor_tensor(out=o1[:sw, :], in0=o_ps[:sw, :],
                                            in1=bb_rep[:sw, :],
                                            op=mybir.AluOpType.add)
                    o2 = wk.tile([128, 3], F32, tag="o2")
                    nc.vector.tensor_tensor(out=o2[:sw, :], in0=o1[:sw, :],
                                            in1=x_sb[:sw, 0:3],
                                            op=mybir.AluOpType.add)
                    nc.sync.dma_start(out_d[c0 + s0:c0 + s0 + sw, :],
                                      o2[:sw, :])
    nc.compile()
    return nc


def _get_nc():
    if "nc" not in _CACHE:
        _CACHE["nc"] = _build()
    return _CACHE["nc"]


def _prep_in_maps(x, edge_index, W1, b1, W2, b2, W3, b3, W4, b4, W5, b5,
                  Wa, ba, Wb, bb):
    x = np.asarray(x, dtype=np.float32)
    src = np.asarray(edge_index[0]).astype(np.int32).reshape(N, K)
    dst = np.asarray(edge_index[1]).astype(np.int64)
    assert np.array_equal(dst, np.repeat(np.arange(N, dtype=np.int64), K)), \
        "kernel assumes dst = repeat(arange(N), K)"
    common = {
        "W1": np.asarray(W1, np.float32), "b1": np.asarray(b1, np.float32),
        "W2": np.asarray(W2, np.float32), "b2": np.asarray(b2, np.float32),
        "W3": np.asarray(W3, np.float32), "b3": np.asarray(b3, np.float32),
        "W4": np.asarray(W4, np.float32), "b4": np.asarray(b4, np.float32),
        "W5": np.asarray(W5, np.float32), "b5": np.asarray(b5, np.float32),
        "Wa": np.asarray(Wa, np.float32), "ba": np.asarray(ba, np.float32),
        "Wb": np.asarray(Wb, np.float32), "bb": np.asarray(bb, np.float32),
    }
    in_maps = []
    for r in range(NCORES):
        lo = NLOC * r
        xp = np.zeros((NPAD, 4), dtype=np.float32)
        xp[:NLOC, 0:3] = x[lo:lo + NLOC]
        xp[:, 3] = 1.0
        sp = np.zeros((NPAD, K), dtype=np.int32)
        sp[:NLOC] = src[lo:lo + NLOC]
        idx_arr = np.zeros((128, NT * K), dtype=np.int32)
        for t in range(NT):
            idx_arr[:, K * t:K * (t + 1)] = sp[128 * t:128 * (t + 1)]
        in_maps.append({"x_loc": xp, "idx": idx_arr, **common})
    return in_maps


def kernel(**inputs) -> np.ndarray:
    from concourse.bass_utils import run_bass_kernel_spmd
    nc = _get_nc()
    in_maps = _prep_in_maps(**inputs)
    res = run_bass_kernel_spmd(nc, in_maps, core_ids=list(range(NCORES)))
    return np.concatenate([res.results[r]["out"] for r in range(NCORES)],
                          axis=0)
